# revision 48
# baseline (speedup 1.0000x reference)
"""Bahdanau-style additive attention on 8 TRN2 NeuronCores.

Reference computation (B=32, S=2048, H=1024):
    query  = hidden @ Wq.T                      # (B, H)
    keys   = enc @ Wk.T                         # (B, S, H)
    energy = tanh(query[:, None, :] + keys)     # (B, S, H)
    attn   = energy @ v                         # (B, S)
    out    = softmax(mask(attn, lengths))       # (B, S)

Sharding: data-parallel, 4 batches per core, no collectives.

Length-aware tile skipping: positions s >= lengths[b] contribute
exactly 0 to the output (softmax of -inf), so seq tiles that are fully
masked are never computed, DMA'd, or stored — the host fills those
output columns with zeros.  SPMD needs one program for all 8 cores, so
batches are sorted by length (descending) and dealt round-robin into 4
per-core slots; slot j's tile count is ceil(max-length-in-rank-octile-j
/ 512), baked into the program (rebuilt+cached per tile-count tuple).
The shortest slot runs last so the final softmax tail drains a short
row.

Per-core dataflow (all matmuls in fp32r mode — 1 cycle/row on the PE):
    - enc is fed pre-transposed (H, S) so the H contraction sits on SBUF
      partitions; tiles are [128h x 512s].
    - keys tile [128f, 512s] accumulates 8 h-chunk matmuls in PSUM.
    - ACT applies tanh with the per-partition query bias, PSUM -> SBUF.
    - PE contracts the energy tile with v ([128,1] stationary) into a
      [1, 512] PSUM accumulator over the 8 f-chunks.
    - DVE adds the (host-precomputed) length-mask bias and keeps a
      running per-batch max; the per-batch softmax tail is ACT exp with
      accumulate, then the normalize runs split across DVE and ACT.
"""

import sys

if "/opt/trn_rl_repo" not in sys.path:
    sys.path.insert(0, "/opt/trn_rl_repo")

import numpy as np

B, S, H = 32, 2048, 1024
NCORES = 8
BPC = B // NCORES  # batches per core
FT = 128           # partition tile (feature / h chunk)
HC = H // FT       # h chunks
ST = 512           # seq tile
NST = S // ST

_CACHE = {}


def _build(nsts=(NST,) * BPC, variant="full", loop_r=1):
    import concourse.bass as bass  # noqa: F401
    import concourse.tile as tile
    from concourse import bacc, mybir

    f32 = mybir.dt.float32
    f32r = mybir.dt.float32r
    Tanh = mybir.ActivationFunctionType.Tanh
    Exp = mybir.ActivationFunctionType.Exp

    nc = bacc.Bacc("TRN2", target_bir_lowering=False, debug=False,
                   num_devices=NCORES)

    encT = nc.dram_tensor("encT", [BPC, H, S], f32r, kind="ExternalInput").ap()
    hT = nc.dram_tensor("hT", [H, BPC], f32r, kind="ExternalInput").ap()
    wkT = nc.dram_tensor("wkT", [H, H], f32r, kind="ExternalInput").ap()
    wqT = nc.dram_tensor("wqT", [H, H], f32r, kind="ExternalInput").ap()
    vp = nc.dram_tensor("vp", [FT, HC], f32r, kind="ExternalInput").ap()
    # mask rows live at partition 32*b so engine APs stay 32-aligned
    mb = nc.dram_tensor("mb", [FT, S], f32, kind="ExternalInput").ap()
    out = nc.dram_tensor("out", [BPC, S], f32, kind="ExternalOutput").ap()

    nraw = min(3, nsts[0]) if loop_r == 1 else 0

    with tile.TileContext(nc) as tc:
        with (
            tc.tile_pool(name="singles", bufs=1) as singles,
            tc.tile_pool(name="encp", bufs=4) as encp,
            tc.tile_pool(name="energy", bufs=4) as ep,
            tc.tile_pool(name="kpsum", bufs=5, space="PSUM") as kps,
            tc.tile_pool(name="apsum", bufs=3, space="PSUM") as aps,
            tc.tile_pool(name="stats", bufs=1) as stats,
        ):
            wk_sb = singles.tile([FT, HC, H], f32r)
            wq_sb = singles.tile([FT, HC, H], f32r)
            ht_sb = singles.tile([FT, HC, BPC], f32r)
            v_sb = singles.tile([FT, HC], f32r)
            mask_sb = singles.tile([FT, S], f32)
            qT_sb = singles.tile([FT, HC, BPC], f32)
            attn_sb = singles.tile([FT, S], f32)
            nm_sb = singles.tile([FT, 1], f32)
            nc.vector.memset(attn_sb[:], 0.0)
            nc.vector.memset(nm_sb[:], -3.0e38)
            enr = (singles.tile([FT, nraw * HC, ST], f32r, name="enr")
                   if nraw else None)

            def emit_q():
                # qT[f, b] = sum_h WqT[h, f] * hiddenT[h, b].  All 8 fc
                # accumulation groups share one PSUM tile: `start` only
                # on the global first matmul (per-element has_written
                # handles first-touch for the other regions; nothing
                # reads the tile until after the global stop).
                qp = aps.tile([FT, HC, BPC], f32, tag="ap")
                for hc in range(HC):
                    for fc in range(HC):
                        nc.tensor.matmul(
                            qp[:, fc, :],
                            lhsT=wq_sb[:, hc, fc * FT:(fc + 1) * FT],
                            rhs=ht_sb[:, hc, :],
                            start=(hc == 0 and fc == 0),
                            stop=(hc == HC - 1 and fc == HC - 1),
                            skip_group_check=True)
                nc.vector.tensor_copy(out=qT_sb[:], in_=qp[:])

            def load_wq():
                for hc in range(HC):
                    nc.sync.dma_start(out=wq_sb[:, hc, :],
                                      in_=wqT[hc * FT:(hc + 1) * FT, :])
                    nc.sync.dma_start(out=ht_sb[:, hc, :],
                                      in_=hT[hc * FT:(hc + 1) * FT, :])

            def load_et(b, st):
                et = encp.tile([FT, HC, ST], f32r, tag="et", name="et")
                for hc in range(HC):
                    nc.sync.dma_start(
                        out=et[:, hc, :],
                        in_=encT[b, hc * FT:(hc + 1) * FT,
                                 st * ST:(st + 1) * ST])
                return et

            def load_et_wide(wt):
                et = encp.tile([FT, HC, 2 * ST], f32r, tag="etw",
                               name="etw", bufs=2)
                for hc in range(HC):
                    nc.sync.dma_start(
                        out=et[:, hc, :],
                        in_=encT[0, hc * FT:(hc + 1) * FT, 0:2 * ST])
                return et

            def load_wk_col(fc):
                nc.sync.dma_start(
                    out=wk_sb[:, :, fc * FT:(fc + 1) * FT],
                    in_=wkT[:, fc * FT:(fc + 1) * FT].rearrange(
                        "(hc p) f -> p hc f", p=FT))

            raw_ets = []
            if nraw:
                # DMA order: Wk column 0 in per-hc 64KB blocks
                # interleaved with the et0 chunks (the first keys matmul
                # needs only block hc=0, so the PE starts at ~1.5us),
                # remaining Wk columns (keys group fc only needs col
                # fc), other raw tiles, Wq LAST — the PE has ~40us of
                # keys work queued by the time q runs.
                et0 = encp.tile([FT, HC, ST], f32r, tag="et", name="et_raw")
                raw_ets.append(et0)
                for hc in range(HC):
                    nc.sync.dma_start(
                        out=wk_sb[:, hc, 0:FT],
                        in_=wkT[hc * FT:(hc + 1) * FT, 0:FT])
                    nc.sync.dma_start(
                        out=et0[:, hc, :],
                        in_=encT[0, hc * FT:(hc + 1) * FT, 0:ST])
                for fc in range(1, HC):
                    load_wk_col(fc)
                for st in range(1, nraw):
                    raw_ets.append(load_et(0, st))
                load_wq()
            else:
                load_wq()
                emit_q()
                for hc in range(HC):
                    nc.sync.dma_start(out=wk_sb[:, hc, :],
                                      in_=wkT[hc * FT:(hc + 1) * FT, :])
            nc.sync.dma_start(out=v_sb[:], in_=vp[:])
            nc.sync.dma_start(out=mask_sb[:], in_=mb[:])

            load_et.wide = load_et_wide
            args = (nc, tc, mybir, f32, f32r, Tanh, Exp, variant, nsts,
                    load_et, out, ep, kps, aps, stats,
                    wk_sb, v_sb, mask_sb, qT_sb, attn_sb, nm_sb)
            if loop_r > 1:
                with tc.For_i(0, loop_r, 1):
                    _body(*args, [], None, None)
            else:
                _body(*args, raw_ets, enr, emit_q)

    nc.compile()
    return nc


def _body(nc, tc, mybir, f32, f32r, Tanh, Exp, variant, nsts,
          load_et, out, ep, kps, aps, stats,
          wk_sb, v_sb, mask_sb, qT_sb, attn_sb, nm_sb, raw_ets, enr,
          emit_q):
    """Emit the main (b, st) tile loop.

    Startup: the first `nraw` seq-tiles of batch 0 run in "raw" mode —
    keys PSUM is drained to SBUF with a plain copy (no q dependency) so
    the PE streams matmuls from t=0 while Wq loads last; tanh runs
    in-place once q is ready and the deferred v-matvecs drain as gap
    fillers (at most 4 per flush point) inside later keys groups.

    Steady state: the v-matvec for a finished energy tile is emitted one
    PE group late (after matmul hc==4 of the next keys group) so the
    tanh latency never stalls the in-order PE queue.  A running
    per-batch max is maintained after each mask-add so the final
    softmax skips the full-row max reduce.
    """
    nraw = len(raw_ets)
    pending_v = []  # (ap_tile, energy_ap, fc, b, st)

    def softmax_row(b, negmax):
        L = nsts[b] * ST
        r0 = 32 * b
        row = attn_sb[r0:r0 + 1, 0:L]
        denom = stats.tile([1, 1], f32, tag="dn", bufs=2)
        nc.scalar.activation(out=row, in_=row, func=Exp,
                             bias=negmax[:], scale=1.0,
                             accum_out=denom[:])
        recip = stats.tile([1, 1], f32, tag="rc", bufs=2)
        nc.vector.reciprocal(out=recip[:], in_=denom[:])
        # normalize halves on DVE and ACT in parallel
        half = L // 2
        rowa = attn_sb[r0:r0 + 1, 0:half]
        rowb = attn_sb[r0:r0 + 1, half:L]
        nc.vector.tensor_scalar_mul(rowa, rowa, recip[:])
        nc.sync.dma_start(out=out[b:b + 1, 0:half], in_=rowa)
        nc.scalar.activation(out=rowb, in_=rowb,
                             func=mybir.ActivationFunctionType.Copy,
                             bias=0.0, scale=recip[:])
        nc.sync.dma_start(out=out[b:b + 1, half:L], in_=rowb)

    def flush(n):
        for _ in range(min(n, len(pending_v))):
            pap, pen, pfc, pb, pst = pending_v.pop(0)
            nc.tensor.matmul(
                pap[:], lhsT=v_sb[:, pfc:pfc + 1], rhs=pen,
                start=(pfc == 0), stop=(pfc == HC - 1))
            if pfc == HC - 1:
                r0 = 32 * pb
                sl = slice(pst * ST, (pst + 1) * ST)
                nc.vector.tensor_add(
                    out=attn_sb[r0:r0 + 1, sl],
                    in0=pap[:], in1=mask_sb[r0:r0 + 1, sl])
                cmax = stats.tile([1, 1], f32, tag="cm", bufs=2)
                nc.vector.tensor_reduce(
                    out=cmax[:], in_=attn_sb[r0:r0 + 1, sl],
                    axis=mybir.AxisListType.X, op=mybir.AluOpType.max)
                if pst < nsts[pb] - 1:
                    nc.vector.tensor_scalar_max(
                        nm_sb[r0:r0 + 1, :], nm_sb[r0:r0 + 1, :], cmax[:])
                else:
                    # last chunk of the batch: fuse the final max-update
                    # with the negation the exp bias needs
                    negmax = stats.tile([1, 1], f32, tag="nm", bufs=2)
                    nc.vector.tensor_scalar(
                        negmax[:], cmax[:], nm_sb[r0:r0 + 1, :], -1.0,
                        mybir.AluOpType.max, mybir.AluOpType.mult)
                    if variant != "noSoftmax":
                        softmax_row(pb, negmax)

    def keys_group(et, fc):
        kp = kps.tile([FT, ST], f32, tag="kp")
        for hc in range(HC):
            nc.tensor.matmul(
                kp[:],
                lhsT=(wk_sb[:, 0, 0:FT] if variant == "samew"
                      else wk_sb[:, hc, fc * FT:(fc + 1) * FT]),
                rhs=et[:, hc, :],
                start=(hc == 0), stop=(hc == HC - 1))
            if hc in (4, 7):
                flush(2)
        return kp

    # ---- raw startup tiles (b=0, st<nraw) ----
    state = {"q": False, "tanhed": 0}

    def raw_tanh_upto(limit):
        for st in range(state["tanhed"], limit):
            ap_ = aps.tile([1, ST], f32, tag="ap")
            for fc in range(HC):
                en = enr[:, st * HC + fc, :]
                nc.scalar.activation(out=en, in_=en, func=Tanh,
                                     bias=qT_sb[:, fc, 0:1], scale=1.0)
                pending_v.append((ap_, en, fc, 0, st))
        state["tanhed"] = limit

    for st in range(nraw):
        for fc in range(HC):
            kp = keys_group(raw_ets[st], fc)
            nc.vector.tensor_copy(out=enr[:, st * HC + fc, :], in_=kp[:])
        if st == 1 and emit_q is not None:
            # Wq has landed by now; running q here lets ACT chew the
            # deferred tanh backlog during the remaining raw keys
            emit_q()
            state["q"] = True
            raw_tanh_upto(2)
    if raw_ets:
        if not state["q"] and emit_q is not None:
            emit_q()
        raw_tanh_upto(nraw)

    if variant == "wide":
        # probe: same total rows as keysonly, half the matmul count
        nwide = (sum(nsts) * ST) // (2 * ST)
        for wt in range(nwide):
            et = load_et.wide(wt)
            for fc in range(HC):
                kp = kps.tile([FT, 2 * ST], f32, tag="kpw", bufs=2)
                for hc in range(HC):
                    nc.tensor.matmul(
                        kp[:],
                        lhsT=wk_sb[:, hc, fc * FT:(fc + 1) * FT],
                        rhs=et[:, hc, :],
                        start=(hc == 0), stop=(hc == HC - 1))
        return

    # ---- steady-state tiles ----
    for b in range(BPC):
        for st in range(nsts[b]):
            if b == 0 and st < nraw:
                continue
            if variant == "nodma":
                if state.get("et0") is None:
                    state["et0"] = load_et(b, st)
                for fc in range(HC):
                    keys_group(state["et0"], fc)
                continue
            if variant == "bf16k":
                bf16 = mybir.dt.bfloat16
                if state.get("et0") is None:
                    et0 = load_et(b, st)
                    etb = ep.tile([FT, HC, ST], bf16, tag="etb", bufs=1)
                    wkb = ep.tile([FT, HC, H], bf16, tag="wkb", bufs=1)
                    nc.vector.tensor_copy(out=etb[:], in_=et0[:])
                    nc.vector.tensor_copy(out=wkb[:], in_=wk_sb[:])
                    state["et0"] = etb
                    state["wkb"] = wkb
                etb, wkb = state["et0"], state["wkb"]
                for fc in range(HC):
                    kp = kps.tile([FT, ST], f32, tag="kp")
                    for hc in range(HC):
                        nc.tensor.matmul(
                            kp[:],
                            lhsT=wkb[:, hc, fc * FT:(fc + 1) * FT],
                            rhs=etb[:, hc, :],
                            start=(hc == 0), stop=(hc == HC - 1))
                continue
            et = load_et(b, st)
            if variant == "keysonly":
                for fc in range(HC):
                    keys_group(et, fc)
                continue
            ap_ = aps.tile([1, ST], f32, tag="ap")
            for fc in range(HC):
                kp = keys_group(et, fc)
                en = ep.tile([FT, ST], f32r, tag="en")
                nc.scalar.activation(
                    out=en[:], in_=kp[:], func=Tanh,
                    bias=qT_sb[:, fc, b:b + 1], scale=1.0)
                pending_v.append((ap_, en[:], fc, b, st))
    flush(len(pending_v))

    if variant == "noSoftmax":
        rawt = stats.tile([BPC, S], f32)
        nc.vector.tensor_copy(out=rawt[:], in_=attn_sb[0:BPC, :])
        nc.sync.dma_start(out=out[:], in_=rawt[:])


MAXC = NST * 4          # max attn columns per batch (s-chunks of 128)


def _build_tv(nsts, loop_r=1, stage=0):
    """Transposed-keys dataflow: PSUM keys tiles are [128s, 512k], so the
    q-add, tanh, and v-weighted reduction run on DVE/ACT along the free
    dim and the PE does nothing but keys matmuls (plus a handful of tiny
    transpose/broadcast matmuls for the per-batch softmax).

    Per (batch, seq-tile): 4 s-chunks x 2 k-halves; each [128,512] PSUM
    tile accumulates 8 h-chunk matmuls, then
        DVE:  en = kt + qb          (q broadcast along partitions)
        ACT:  en = tanh(en)         (in place)
        DVE:  ttr: prod = en * vb, attn_col = reduce(prod) chained
    The per-batch softmax reduces across partitions with PE transpose
    (matmul by identity) and broadcast (ones-column matmul) tricks.
    """
    import concourse.bass as bass  # noqa: F401
    import concourse.tile as tile
    from concourse import bacc, mybir

    f32 = mybir.dt.float32
    f32r = mybir.dt.float32r
    Tanh = mybir.ActivationFunctionType.Tanh
    Exp = mybir.ActivationFunctionType.Exp
    Copy = mybir.ActivationFunctionType.Copy
    A = mybir.AluOpType
    X = mybir.AxisListType.X

    nc = bacc.Bacc("TRN2", target_bir_lowering=False, debug=False,
                   num_devices=NCORES)

    encT = nc.dram_tensor("encT", [BPC, H, S], f32r, kind="ExternalInput").ap()
    hT = nc.dram_tensor("hT", [H, BPC], f32r, kind="ExternalInput").ap()
    wkT = nc.dram_tensor("wkT", [H, H], f32r, kind="ExternalInput").ap()
    wqT = nc.dram_tensor("wqT", [H, H], f32r, kind="ExternalInput").ap()
    identf = nc.dram_tensor("identf", [FT, FT], f32r,
                            kind="ExternalInput").ap()
    onesf = nc.dram_tensor("onesf", [FT, FT], f32, kind="ExternalInput").ap()
    vbb = nc.dram_tensor("vbb", [FT, H], f32, kind="ExternalInput").ap()
    selmf = nc.dram_tensor("selm", [BPC, BPC * FT], f32r,
                           kind="ExternalInput").ap()
    mcols = nc.dram_tensor("mcols", [FT, BPC * MAXC], f32,
                           kind="ExternalInput").ap()
    out = nc.dram_tensor("out", [BPC, S], f32, kind="ExternalOutput").ap()
    outv = out.rearrange("b (c p) -> b p c", p=FT)

    with tile.TileContext(nc) as tc:
        with (
            tc.tile_pool(name="singles", bufs=1) as singles,
            tc.tile_pool(name="encp", bufs=4) as encp,
            tc.tile_pool(name="enp", bufs=4) as enp,
            tc.tile_pool(name="qbp", bufs=2) as qbp,
            tc.tile_pool(name="colp", bufs=2) as colp,
            tc.tile_pool(name="stats", bufs=1) as stats,
            tc.tile_pool(name="kpsum", bufs=5, space="PSUM") as kps,
            tc.tile_pool(name="mpsum", bufs=1, space="PSUM") as mps,
        ):
            wk_sb = singles.tile([FT, HC, H], f32r)
            wq_sb = singles.tile([FT, HC, H], f32r)
            ht_sb = singles.tile([FT, HC, BPC], f32r)
            ident = singles.tile([FT, FT], f32r)
            onesm = singles.tile([FT, FT], f32)
            nmcol = singles.tile([FT, 2], f32)
            psc2 = singles.tile([FT, 2], f32)
            nc.vector.memset(nmcol[:], 0.0)
            nc.vector.memset(psc2[:], 0.0)
            vb_sb = singles.tile([FT, H], f32)
            mc_sb = singles.tile([FT, BPC * MAXC], f32)
            qrow_sb = singles.tile([BPC, H], f32r)

            for hc in range(HC):
                nc.sync.dma_start(out=wq_sb[:, hc, :],
                                  in_=wqT[hc * FT:(hc + 1) * FT, :])
                nc.sync.dma_start(out=ht_sb[:, hc, :],
                                  in_=hT[hc * FT:(hc + 1) * FT, :])
            for hc in range(HC):
                nc.sync.dma_start(out=wk_sb[:, hc, :],
                                  in_=wkT[hc * FT:(hc + 1) * FT, :])
            nc.sync.dma_start(out=ident[:], in_=identf[:])
            nc.sync.dma_start(out=onesm[:], in_=onesf[:])
            nc.sync.dma_start(out=vb_sb[:], in_=vbb[:])
            nc.sync.dma_start(out=mc_sb[:], in_=mcols[:])
            selm = singles.tile([BPC, BPC * FT], f32r)
            nc.sync.dma_start(out=selm[:], in_=selmf[:])

            # q row per batch: qrow[b, k] = sum_h hidden[b, h] Wq[k, h]
            for kh in range(2):
                qp = kps.tile([FT, ST], f32, tag="kt")
                for hc in range(HC):
                    nc.tensor.matmul(
                        qp[0:BPC, :],
                        lhsT=ht_sb[:, hc, :],
                        rhs=wq_sb[:, hc, kh * ST:(kh + 1) * ST],
                        start=(hc == 0), stop=(hc == HC - 1))
                nc.vector.tensor_copy(
                    out=qrow_sb[:, kh * ST:(kh + 1) * ST], in_=qp[0:BPC, :])

            def emit_qb(b):
                # qb[p, k] = qrow[b, k] for all p: select row b via a
                # [4, 128] one-hot stationary (keeps AP bases at 0)
                qb = qbp.tile([FT, H], f32, tag="qb")
                for kh in range(2):
                    qbps = kps.tile([FT, ST], f32, tag="kt")
                    nc.tensor.matmul(
                        qbps[:], lhsT=selm[:, b * FT:(b + 1) * FT],
                        rhs=qrow_sb[:, kh * ST:(kh + 1) * ST],
                        start=True, stop=True)
                    nc.vector.tensor_copy(out=qb[:, kh * ST:(kh + 1) * ST],
                                          in_=qbps[:])
                return qb

            def do_tile(b, st, qb, attn_cols):
                et = encp.tile([FT, HC, ST], f32r, tag="et", name="et")
                for hc in range(HC):
                    nc.sync.dma_start(
                        out=et[:, hc, :],
                        in_=encT[b, hc * FT:(hc + 1) * FT,
                                 st * ST:(st + 1) * ST])
                for sc in range(4):
                    col = st * 4 + sc
                    prod = enp.tile([FT, 2 * ST], f32, tag="prod", bufs=2)
                    for kh in range(2):
                        kt = kps.tile([FT, ST], f32, tag="kt")
                        for hc in range(HC):
                            nc.tensor.matmul(
                                kt[:],
                                lhsT=et[:, hc, sc * FT:(sc + 1) * FT],
                                rhs=wk_sb[:, hc, kh * ST:(kh + 1) * ST],
                                start=(hc == 0), stop=(hc == HC - 1))
                        if stage >= 4:
                            continue
                        en = enp.tile([FT, ST], f32, tag="en")
                        nc.vector.tensor_add(out=en[:], in0=kt[:],
                                             in1=qb[:, kh * ST:(kh + 1) * ST])
                        nc.scalar.activation(out=en[:], in_=en[:], func=Tanh,
                                             bias=0.0, scale=1.0)
                        if stage >= 3:
                            continue
                        nc.vector.tensor_tensor(
                            out=prod[:, kh * ST:(kh + 1) * ST], in0=en[:],
                            in1=vb_sb[:, kh * ST:(kh + 1) * ST], op=A.mult)
                    if stage < 3:
                        nc.vector.tensor_reduce(
                            out=attn_cols[:, col:col + 1], in_=prod[:],
                            axis=X, op=A.add)

            def softmax_b(b, attn_cols, ncols):
                am = colp.tile([FT, MAXC], f32, tag="am")
                pmax = stats.tile([FT, 1], f32r, tag="pmax", bufs=2)
                nc.vector.tensor_add(out=am[:, 0:ncols],
                                     in0=attn_cols[:, 0:ncols],
                                     in1=mc_sb[:, b * MAXC:b * MAXC + ncols])
                nc.vector.tensor_reduce(out=pmax[:], in_=am[:, 0:ncols],
                                        axis=X, op=A.max)
                # row max: transpose pmax via PE, free-reduce on DVE into
                # partition 0 of the pre-zeroed staging column, then an
                # all-ones matmul (contraction 128) re-broadcasts it
                tp = mps.tile([1, FT], f32, tag="tp")
                nc.tensor.matmul(tp[:], lhsT=pmax[:], rhs=ident[:],
                                 start=True, stop=True)
                nc.vector.tensor_reduce(out=nmcol[0:1, 0:1], in_=tp[:],
                                        axis=X, op=A.max)
                bcm = mps.tile([FT, 2], f32, tag="bc")
                nc.tensor.matmul(bcm[:], lhsT=onesm[:], rhs=nmcol[:],
                                 start=True, stop=True)
                ngm = stats.tile([FT, 1], f32, tag="ngm", bufs=2)
                nc.vector.tensor_scalar_mul(ngm[:], bcm[:, 0:1], -1.0)
                probs = colp.tile([FT, MAXC], f32, tag="pr")
                nc.scalar.activation(out=probs[:, 0:ncols],
                                     in_=am[:, 0:ncols], func=Exp,
                                     bias=ngm[:], scale=1.0,
                                     accum_out=psc2[:, 0:1])
                # denominator: all-ones matmul sums the per-partition exp
                # accumulators across partitions and broadcasts in one go
                den = mps.tile([FT, 2], f32, tag="bc")
                nc.tensor.matmul(den[:], lhsT=onesm[:], rhs=psc2[:],
                                 start=True, stop=True)
                recb = stats.tile([FT, 1], f32, tag="recb", bufs=2)
                nc.vector.reciprocal(out=recb[:], in_=den[:, 0:1])
                nc.scalar.activation(out=probs[:, 0:ncols],
                                     in_=probs[:, 0:ncols], func=Copy,
                                     bias=0.0, scale=recb[:])
                if stage >= 1:
                    return
                nc.sync.dma_start(out=outv[b, :, 0:ncols],
                                  in_=probs[:, 0:ncols])

            def body():
                for b in range(BPC):
                    qb = emit_qb(b)
                    attn_cols = colp.tile([FT, MAXC], f32, tag="ac")
                    for st in range(nsts[b]):
                        do_tile(b, st, qb, attn_cols)
                    if stage < 2:
                        softmax_b(b, attn_cols, nsts[b] * 4)

            if loop_r > 1:
                with tc.For_i(0, loop_r, 1):
                    body()
            else:
                body()

    nc.compile()
    return nc


def _get_nc(nsts):
    key = ("nc_tv", tuple(nsts))
    if key not in _CACHE:
        _CACHE[key] = _build_tv(tuple(nsts))
    return _CACHE[key]


def _plan(lengths):
    """Deal batches (sorted by length, descending) round-robin into the
    4 per-core slots; slot tile counts come from each rank-octile max."""
    lengths = np.asarray(lengths).astype(np.int64)
    order = np.argsort(-lengths, kind="stable")
    slots = order.reshape(BPC, NCORES)        # slots[j, c] -> batch index
    nsts = tuple(int(np.ceil(lengths[slots[j]].max() / ST))
                 for j in range(BPC))
    return slots, nsts


def _prepare_in_maps(hidden, encoder_outputs, lengths, Wq, Wk, v):
    hidden = np.ascontiguousarray(np.asarray(hidden, dtype=np.float32))
    enc = np.asarray(encoder_outputs, dtype=np.float32)
    lengths = np.asarray(lengths).astype(np.int64)
    Wq = np.asarray(Wq, dtype=np.float32)
    Wk = np.asarray(Wk, dtype=np.float32)
    v = np.asarray(v, dtype=np.float32)

    slots, nsts = _plan(lengths)

    hiddenT = np.ascontiguousarray(hidden.T)                     # (H, B)
    WkT = np.ascontiguousarray(Wk.T)                             # (H, H)
    WqT = np.ascontiguousarray(Wq.T)                             # (H, H)
    vp = np.ascontiguousarray(v.reshape(HC, FT).T)               # (128, 8)
    maskb = np.where(np.arange(S)[None, :] < lengths[:, None],
                     np.float32(0.0), np.float32(-1e30)).astype(np.float32)
    # pad to 128 partitions: slot j of the core sits at row 32*j
    maskp = np.zeros((NCORES, FT, S), dtype=np.float32)
    for c in range(NCORES):
        for j in range(BPC):
            maskp[c, 32 * j] = maskb[slots[j, c]]

    # column-layout mask for the tv build: mcols[p, j*MAXC + c] is the
    # bias for s = c*128 + p of the core's slot-j batch
    identf = np.eye(FT, dtype=np.float32)
    vbbc = np.ascontiguousarray(np.broadcast_to(v, (FT, H)))
    selm = np.zeros((BPC, BPC * FT), dtype=np.float32)
    for j in range(BPC):
        selm[j, j * FT:(j + 1) * FT] = 1.0
    mcolsp = np.zeros((NCORES, FT, BPC * MAXC), dtype=np.float32)
    for c in range(NCORES):
        for j in range(BPC):
            m = maskb[slots[j, c]].reshape(MAXC, FT).T    # [128, 16]
            mcolsp[c, :, j * MAXC:(j + 1) * MAXC] = m

    in_maps = []
    for c in range(NCORES):
        bs = slots[:, c]
        in_maps.append({
            "encT": np.ascontiguousarray(enc[bs].transpose(0, 2, 1)),
            "hT": np.ascontiguousarray(hiddenT[:, bs]),
            "wkT": WkT,
            "wqT": WqT,
            "vp": vp,
            "mb": maskp[c],
            "identf": identf,
            "onesf": np.ones((FT, FT), dtype=np.float32),
            "vbb": vbbc,
            "selm": selm,
            "mcols": mcolsp[c],
        })
    return in_maps, slots, nsts


def _run(in_maps, nsts, trace=False, **kw):
    from concourse.bass_utils import run_bass_kernel_spmd
    nc = _get_nc(nsts)
    res = run_bass_kernel_spmd(nc, in_maps, core_ids=list(range(NCORES)),
                               trace=trace, **kw)
    return res


def _assemble(res, slots, nsts):
    out = np.zeros((B, S), dtype=np.float32)
    for c in range(NCORES):
        for j in range(BPC):
            L = nsts[j] * ST
            out[slots[j, c], :L] = res.results[c]["out"][j, :L]
    return out


def kernel(hidden, encoder_outputs, lengths, Wq, Wk, v):
    in_maps, slots, nsts = _prepare_in_maps(hidden, encoder_outputs,
                                            lengths, Wq, Wk, v)
    res = _run(in_maps, nsts, trace=False)
    return _assemble(res, slots, nsts)

